# revision 24
# baseline (speedup 1.0000x reference)
"""GAT-style 3-layer attention graph network on 8 TRN2 NeuronCores.

Math: per layer, alpha[i,j] = adj[i,j]*exp(el[i]+er[j]+ab) / sum_k adj[i,k]*exp(el[i]+er[k]+ab)
The exp(el[i]) factor cancels between numerator and denominator, so with
w[j] = exp(er[j]+ab):
    out[i] = relu( (sum_j adj[i,j]*w[j]*h[j]) / (sum_j adj[i,j]*w[j]) )
i.e. one [N,N]@[N,F+1] matmul per layer against G = [h*w | w], with adj
constant across layers.

Distribution: row-shard adj across the 8 cores (1024 dest rows each). adj is
0/1 so it is exactly representable in fp8_e4m3. Layers 0 and 1 run the
aggregation as fp8 DoubleRow matmuls (G stored fp8_e4m3, 2 contraction rows
per PE pass -> 2x): lhsT is [128, 2, 64] (256-row contraction, 64 dest rows),
rhs [128, 2, F+1]. Layer 2 keeps G in fp16 (fp8 G there pushes rel err past
the gate) using the same adj tile via a strided [2, 64] lhsT AP.
The host pre-transposes each core's adj row-block into a single layout
[p, m, kp, i, h, q] fp8 that serves both paths; it stays SBUF-resident
(8MB/core) across all 3 layers.
Each layer all-gathers the 8192x(F+1) G matrix (fp8 for layers 0/1, fp16 for
layer 2) in two node-halves so the first gather overlaps the previous
aggregation; G blocks are staged to DRAM per 128-node block as soon as each
is built so only the collective + SBUF reload sit on the layer boundary.
"""
import numpy as np

import concourse.bass as bass
import concourse.mybir as mybir
import concourse.tile as tile
from concourse.masks import make_identity
from concourse.bass_utils import run_bass_kernel_spmd

F32 = mybir.dt.float32
F16 = mybir.dt.float16
F8 = mybir.dt.float8e4    # adj storage + layers 0/1 G: e4m3

N_CORES = 8
N = 8192
NL = N // N_CORES          # 1024 local dest rows per core
NT = NL // 128             # 8 local node tiles
KT = N // 128              # 64 contraction tiles (fp16 path)
KP = KT // 2               # 32 contraction pair-tiles (DoubleRow path)
LEAK = 0.2
H1 = 4                     # node-blocks in the first gather half (of NT=8)
SPLITS = [4, 4, 4]         # per-layer first-gather-half block count
DR = mybir.MatmulPerfMode.DoubleRow


def _split_excess_waits(nc, max_waits=1):
    """This walrus build allows only one sync-wait command per instruction;
    split any instruction carrying more into preceding single-wait nops."""
    n_split = 0
    for fn in nc.m.functions:
        for bb in fn.blocks:
            insts = bb.instructions
            i = 0
            while i < len(insts):
                inst = insts[i]
                si = inst.sync_info
                if si is not None and len(si.on_wait) > max_waits:
                    waits = list(si.on_wait)
                    extra, keep = waits[:-max_waits], waits[-max_waits:]
                    nops = []
                    for j, w in enumerate(extra):
                        nop = mybir.InstNoOp(
                            name=f"{inst.name}-waitsplit-{j}", ins=[], outs=[]
                        )
                        nop.engine = inst.engine
                        nop.sync_info = mybir.SyncInfo(on_wait=[w], on_update=[])
                        nops.append(nop)
                    inst.sync_info = mybir.SyncInfo(
                        on_wait=keep, on_update=list(si.on_update)
                    )
                    insts[i:i] = nops
                    i += len(nops)
                    n_split += 1
                i += 1
    return n_split


def _build_program(ab, for_sim=False):
    """ab: the three attention bias floats (baked in as memset constants)."""
    fhs = [128, 128, 64]   # per-layer linear output width
    gdt = [F8, F8, F16]    # per-layer G storage dtype

    nc = bass.Bass(num_devices=N_CORES)

    # adj, host pre-tiled: adjt[p, m, kp, i, h, q] =
    #   adj_local[m*128 + h*64 + q, kp*256 + i*128 + p]
    adj_ext = nc.dram_tensor("adjt", [128, NT, KP, 2, 2, 64], F8,
                             kind="ExternalInput")
    x_ext = nc.dram_tensor("xt_local", [128, NL], F16, kind="ExternalInput")
    # fp16 params: cols [0:128)=W0T [128:256)=W1T [256:320)=W2T, 320+l=awr_l
    par16_ext = nc.dram_tensor("par16", [128, 324], F16, kind="ExternalInput")
    # fp32 params: col l = b_l (rows past fh zero-padded)
    parf_ext = nc.dram_tensor("parf", [128, 4], F32, kind="ExternalInput")
    out_ext = nc.dram_tensor("out", [NL, 64], F32, kind="ExternalOutput")

    # all-gather payload, split in two node-halves per layer: half h of layer
    # l holds rank blocks [128, 4*(fh+1)] with (p, t, f) = G[c*1024+(b0+t)*128+p, f]
    hblk = [[SPLITS[l], NT - SPLITS[l]] for l in range(3)]
    ag_ext = [[nc.dram_tensor(f"ag{l}h{h}",
                              [N_CORES * 128, hblk[l][h] * (fhs[l] + 1)],
                              gdt[l], addr_space="Shared") for h in range(2)]
              for l in range(3)]

    with tile.TileContext(nc) as tc:
        with (
            tc.tile_pool(name="const", bufs=1) as cp,
            tc.tile_pool(name="adjt", bufs=1) as ap_,
            tc.tile_pool(name="slabs", bufs=1) as sp,
            tc.tile_pool(name="gsb", bufs=2) as gp,
            tc.tile_pool(name="misc", bufs=2) as mp,
            tc.tile_pool(name="gloc", bufs=2) as glp,
            tc.tile_pool(name="dram", bufs=3, space="DRAM") as dp,
            tc.tile_pool(name="ptf", bufs=2, space="PSUM") as ptf,
            tc.tile_pool(name="plin", bufs=1, space="PSUM") as plin,
            tc.tile_pool(name="per", bufs=1, space="PSUM") as per,
            tc.tile_pool(name="pbig", bufs=4, space="PSUM") as pbig,
        ):
            # ---- constants / params ----
            ident16 = cp.tile([128, 128], F16)
            make_identity(nc, ident16[:])
            # load order tuned for the layer-0 prologue: W0/awr columns and
            # the first x slab first, everything else after
            par16 = cp.tile([128, 324], F16)
            parf = cp.tile([128, 4], F32)
            curT = sp.tile([128, NL], F16, tag="slab")
            nc.sync.dma_start(out=par16[:, 0:128], in_=par16_ext[:, 0:128])
            nc.sync.dma_start(out=curT[:, 0:256], in_=x_ext[:, 0:256])
            nc.sync.dma_start(out=par16[:, 128:324], in_=par16_ext[:, 128:324])
            nc.sync.dma_start(out=parf[:], in_=parf_ext.ap())
            nc.sync.dma_start(out=curT[:, 256:NL], in_=x_ext[:, 256:NL])
            woff = [0, 128, 256]
            wt_sb = [par16[:, woff[l]:woff[l] + fhs[l]] for l in range(3)]
            awr_sb = [par16[0:fhs[l], 320 + l:321 + l] for l in range(3)]
            b_sb = [parf[0:fhs[l], l:l + 1] for l in range(3)]
            ab_sb = []
            for l in range(3):
                t = cp.tile([128, 1], F32, tag=f"ab{l}")
                nc.gpsimd.memset(t[:], float(ab[l]))
                ab_sb.append(t)

            # ---- adj streams in per node-block chunk; layer-0 m-chains
            # consume chunks as they land. Piece size tuned so the transfer
            # time (~1us) matches the SWDGE generation cadence (~1us): the
            # DMA FIFO stays drained and G-store/gather/reload hops slot in
            # within ~1us instead of queueing behind a backlog ----
            adjT = ap_.tile([128, NT, KP, 2, 2, 64], F8)
            for d in range(NT):
                for k0, k1 in ((0, 11), (11, 22), (22, 32)):
                    nc.gpsimd.dma_start(
                        out=adjT[:, d, k0:k1, :, :, :],
                        in_=adj_ext[:, d, k0:k1, :, :, :])

            # local G staging in DRAM for the all-gather, written per block
            gld = [[dp.tile([128, hblk[l][h] * (fhs[l] + 1)], gdt[l],
                            tag="gld", name=f"gld{l}h{h}")
                    for h in range(2)] for l in range(3)]


            # ---- G-prep helper: one 128-node block of layer l's G ----
            # src_col: [128(fi), 128] fp16 column of transposed prev activations
            def prep_block(l, src_col, gl, m):
                fh = fhs[l]
                pl = plin.tile([128, 256], F32, tag="lin")
                nc.tensor.matmul(pl[0:fh, 0:128], wt_sb[l], src_col,
                                 start=True, stop=True)
                hcol = mp.tile([128, 128], F16, tag="hcol")
                nc.scalar.activation(
                    hcol[0:fh, :], pl[0:fh, 0:128],
                    mybir.ActivationFunctionType.Prelu,
                    bias=b_sb[l], scale=1.0, alpha=LEAK,
                )
                pe_ = per.tile([128, 2], F32, tag="er")
                nc.tensor.matmul(pe_[:, 0:1], hcol[0:fh, :], awr_sb[l],
                                 start=True, stop=True)
                ec = mp.tile([128, 1], F32, tag="expc")
                nc.scalar.activation(
                    ec[:], pe_[:, 0:1], mybir.ActivationFunctionType.Exp,
                    bias=ab_sb[l][:], scale=1.0,
                )
                ptg = ptf.tile([128, 128], F16, tag="ptf")
                nc.tensor.transpose(ptg[:, 0:fh], hcol[0:fh, :],
                                    ident16[0:fh, 0:fh])
                nc.vector.tensor_scalar_mul(gl[:, m, 0:fh], ptg[:, 0:fh], ec[:])
                nc.vector.tensor_copy(gl[:, m, fh:fh + 1], ec[:])

            gsb_tiles = {}

            def fire_gather(l, gl, h):
                """DRAM-stage node-half h of layer l's local G, all-gather
                it, then queue the SBUF reload of that half. One store per
                half (each HWDGE dma costs ~630ns of serial descriptor-gen).
                The two halves get separate SBUF tiles (tile-level deps stay
                independent) and separate DGE queues (SP for half 0, Act for
                half 1) so their store/collective/reload ladders overlap
                instead of serializing on one in-order queue."""
                fh = fhs[l]
                b0 = 0 if h == 0 else SPLITS[l]
                nb = hblk[l][h]
                eng = nc.sync if h == 0 else nc.scalar
                eng.dma_start(out=gld[l][h][:], in_=gl[:, b0:b0 + nb, :])
                if for_sim:
                    # stand-in approximating the collective's wire time
                    # (~7/8 of the half moves over D2D); the real collective
                    # runs on the CC engine and does not consume HWDGE slots
                    eng.dma_start(
                        out=ag_ext[l][h][0:128, :], in_=gld[l][h][:])
                    eng.dma_start(
                        out=ag_ext[l][h][128:256, :], in_=gld[l][h][:])
                else:
                    nc.gpsimd.collective_compute(
                        "AllGather", mybir.AluOpType.bypass,
                        replica_groups=[list(range(N_CORES))],
                        ins=[gld[l][h].opt()], outs=[ag_ext[l][h].ap().opt()],
                    )
                if h == 0:
                    gsb_tiles[l] = (
                        gp.tile([128, SPLITS[l], N_CORES, fhs[l] + 1],
                                gdt[l], tag="gsbA", name="gsba"),
                        gp.tile([128, NT - SPLITS[l], N_CORES, fhs[l] + 1],
                                gdt[l], tag="gsbB", name="gsbb"),
                    )
                return eng.dma_start(
                    out=gsb_tiles[l][h][:, :, :, :],
                    in_=ag_ext[l][h].ap().rearrange(
                        "(c p) (t f) -> p t c f", p=128, f=fh + 1
                    ),
                )

            # ---- layer 0 G from x (overlaps the adj load): pipelined over
            # the four 256-node slabs (all linears issue back-to-back on the
            # then-idle pbig banks, so only two Act round-trips sit on the
            # startup critical path) ----
            gl_cur = glp.tile([128, NT, fhs[0] + 1], F8, tag="gloc")
            fh0 = fhs[0]
            pls = []
            for j in range(4):
                plx = pbig.tile([128, 256], F32, tag="big", name="plx")
                nc.tensor.matmul(plx[0:fh0, 0:256], wt_sb[0],
                                 curT[:, 256 * j:256 * (j + 1)],
                                 start=True, stop=True)
                pls.append(plx)
            hcs = []
            for j in range(4):
                hc = mp.tile([128, 256], F16, tag="hcol2", bufs=4, name="hc")
                if j % 2 == 0:
                    nc.scalar.activation(
                        hc[0:fh0, :], pls[j][0:fh0, 0:256],
                        mybir.ActivationFunctionType.Prelu,
                        bias=b_sb[0], scale=1.0, alpha=LEAK,
                    )
                else:
                    # leaky-relu on DVE so the four prologue activations
                    # split across two engines
                    zt = mp.tile([128, 256], F16, tag="zt", bufs=2, name="zt")
                    nc.vector.tensor_scalar_add(
                        zt[0:fh0, :], pls[j][0:fh0, 0:256], b_sb[0])
                    nc.vector.scalar_tensor_tensor(
                        hc[0:fh0, :], zt[0:fh0, :], LEAK, zt[0:fh0, :],
                        op0=mybir.AluOpType.mult, op1=mybir.AluOpType.max)
                hcs.append(hc)
            ecs = []
            for j in range(4):
                pe_ = per.tile([128, 2], F32, tag="er")
                for i in range(2):
                    nc.tensor.matmul(pe_[:, i:i + 1],
                                     hcs[j][0:fh0, 128 * i:128 * (i + 1)],
                                     awr_sb[0], start=True, stop=True)
                ec = mp.tile([128, 2], F32, tag="expc2", bufs=4, name="ec")
                nc.scalar.activation(
                    ec[:], pe_[:, 0:2], mybir.ActivationFunctionType.Exp,
                    bias=ab_sb[0][:], scale=1.0,
                )
                ecs.append(ec)
            for j in range(4):
                for i in range(2):
                    m = 2 * j + i
                    ptg = ptf.tile([128, 128], F16, tag="ptf")
                    nc.tensor.transpose(ptg[:, 0:fh0],
                                        hcs[j][0:fh0, 128 * i:128 * (i + 1)],
                                        ident16[0:fh0, 0:fh0])
                    nc.vector.tensor_scalar_mul(
                        gl_cur[:, m, 0:fh0], ptg[:, 0:fh0], ecs[j][:, i:i + 1])
                    nc.vector.tensor_copy(
                        gl_cur[:, m, fh0:fh0 + 1], ecs[j][:, i:i + 1])
                if j == 1:
                    fire_gather(0, gl_cur, 0)
            fire_gather(0, gl_cur, 1)

            # ---- layers ----
            def make_epilogue(l, gl_next, ostage):
                fh = fhs[l]

                def epilogue(m, bp):
                    if l < 2:
                        # DoubleRow output: two banks, each h-half in
                        # partitions 0:64 (one accumulation group per bank --
                        # two groups in one bank wedge the runtime)
                        recip = mp.tile([128, 2], F32, tag="recip")
                        for h in range(2):
                            nc.vector.reciprocal(recip[0:64, h:h + 1],
                                                 bp[h][0:64, fh:fh + 1])
                        h2 = mp.tile([64, 256], F16, tag="h2")
                        for h in range(2):
                            # relu(num * recip) on DVE: the Act engine is the
                            # steady-state bottleneck otherwise
                            nc.vector.tensor_scalar(
                                h2[:, 128 * h:128 * (h + 1)],
                                bp[h][0:64, 0:fh],
                                recip[0:64, h:h + 1], 0.0,
                                op0=mybir.AluOpType.mult,
                                op1=mybir.AluOpType.max)
                        pt = ptf.tile([128, 128], F16, tag="ptf")
                        for h in range(2):
                            nc.tensor.transpose(
                                pt[:, 64 * h:64 * (h + 1)],
                                h2[0:64, 128 * h:128 * (h + 1)],
                                ident16[0:64, 0:64])
                        cpcol = mp.tile([128, 128], F16, tag="cpcol")
                        nc.vector.tensor_copy(cpcol[:], pt[:, 0:128])
                        prep_block(l + 1, cpcol[:], gl_next, m)
                        if m == SPLITS[l + 1] - 1:
                            fire_gather(l + 1, gl_next, 0)
                        elif m == NT - 1:
                            fire_gather(l + 1, gl_next, 1)
                    else:
                        recip = mp.tile([128, 2], F32, tag="recip")
                        nc.vector.reciprocal(recip[:, 0:1], bp[:, fh:fh + 1])
                        ot = mp.tile([128, 64], F32, tag="ot")
                        nc.scalar.activation(
                            ot[:], bp[:, 0:fh],
                            mybir.ActivationFunctionType.Relu,
                            bias=0.0, scale=recip[:, 0:1],
                        )
                        nc.sync.dma_start(
                            out=out_ext[m * 128:(m + 1) * 128, :], in_=ot[:])

                return epilogue

            from collections import deque
            pending = deque()

            def flush(n=None):
                k = len(pending) if n is None else min(n, len(pending))
                for _ in range(k):
                    f, pm, pbp = pending.popleft()
                    f(pm, pbp)

            for l in range(3):
                fh = fhs[l]
                gsb = gsb_tiles[l]
                if l < 2:
                    gl_next = glp.tile([128, NT, fhs[l + 1] + 1], gdt[l + 1],
                                       tag="gloc")
                    ostage = None
                else:
                    gl_next = None
                    ostage = True
                epi = make_epilogue(l, gl_next, ostage)

                if l < 2:
                    # DoubleRow items (h, kp); gather-half 0 covers
                    # t0 < SPLITS[l] i.e. kp % 4 < SPLITS[l] // 2
                    sl2 = SPLITS[l] // 2
                    items = [(h, kp) for h in range(2)
                             for kp in range(KP) if kp % 4 < sl2] + \
                            [(h, kp) for h in range(2)
                             for kp in range(KP) if kp % 4 >= sl2]
                    first_i = {}
                    last_i = {}
                    for i, (h, kp) in enumerate(items):
                        first_i.setdefault(h, i)
                        last_i[h] = i
                    n1 = 2 * 8 * sl2  # first-half instruction count

                    def mm_run(bp, m, i0, i1):
                        for i in range(i0, i1):
                            h, kp = items[i]
                            t0 = 2 * (kp % 4)
                            gt, tb = (gsb[0], t0) if t0 < SPLITS[l] else \
                                     (gsb[1], t0 - SPLITS[l])
                            nc.tensor.matmul(
                                bp[h][0:64, 0:fh + 1],
                                adjT[:, m, kp, :, h, :],
                                gt[:, tb:tb + 2, kp // 4, :],
                                start=(i == first_i[h]), stop=(i == last_i[h]),
                                perf_mode=DR,
                            )
                else:
                    # fp16 path; gather-half 0 covers t = k % 8 < 4
                    ks = [k for k in range(KT) if k % NT < SPLITS[l]] + \
                         [k for k in range(KT) if k % NT >= SPLITS[l]]
                    n1 = KT // 2

                    def mm_run(bp, m, i0, i1):
                        for i in range(i0, i1):
                            k = ks[i]
                            t = k % NT
                            gt, tb = (gsb[0], t) if t < SPLITS[l] else \
                                     (gsb[1], t - SPLITS[l])
                            nc.tensor.matmul(
                                bp[:, 0:fh + 1],
                                adjT[:, m, k // 2, k % 2, :, :],
                                gt[:, tb, k // NT, :],
                                start=(i == 0), stop=(i == KT - 1),
                            )

                nk = KT  # total MM instructions per m-chain (both paths)

                def alloc_bp():
                    # one accumulation group per PSUM bank: DoubleRow chains
                    # need a bank pair (the two 64-row h-groups)
                    if l < 2:
                        return (pbig.tile([128, fh + 1], F32, tag="big", name="bpa"),
                                pbig.tile([128, fh + 1], F32, tag="big", name="bpb"))
                    return pbig.tile([128, fh + 1], F32, tag="big", name="bpc")

                # each pending epilogue is flushed right before its banks are
                # reallocated (2 banks/chain for l<2 -> 2-chain pipeline);
                # seam: the previous layer's m=7 epilogue (which fires the
                # G-half-2 gather) lands before any second-half MM
                flush(1)
                bp0 = alloc_bp()
                mm_run(bp0, 0, 0, n1)
                flush()
                bp1 = alloc_bp()
                mm_run(bp1, 1, 0, n1)
                mm_run(bp0, 0, n1, nk)
                mm_run(bp1, 1, n1, nk)
                pending.append((epi, 0, bp0))
                pending.append((epi, 1, bp1))
                for m in range(2, NT):
                    flush(1)
                    bp = alloc_bp()
                    mm_run(bp, m, 0, n1)
                    mm_run(bp, m, n1, nk)
                    pending.append((epi, m, bp))
            flush()

    _split_excess_waits(nc)
    return nc


_PROG_CACHE = {}


def _get_program(ab):
    key = tuple(round(a, 9) for a in ab)
    if key not in _PROG_CACHE:
        _PROG_CACHE[key] = _build_program(ab)
    return _PROG_CACHE[key]


def _make_in_maps(inputs):
    """Build the per-core input maps from the full (unsharded) input dict."""
    import ml_dtypes
    fhs = [128, 128, 64]
    x = np.asarray(inputs["x"], np.float32)
    adj = np.asarray(inputs["adj"], np.float32)

    par16 = np.zeros((128, 324), np.float16)
    parf = np.zeros((128, 4), np.float32)
    woff = [0, 128, 256]
    for l in range(3):
        W = np.asarray(inputs[f"W{l}"], np.float32)
        b = np.asarray(inputs[f"b{l}"], np.float32)
        aW = np.asarray(inputs[f"aW{l}"], np.float32)
        par16[:, woff[l]:woff[l] + fhs[l]] = W.T.astype(np.float16)
        par16[:fhs[l], 320 + l] = aW[0, fhs[l]:2 * fhs[l]].astype(np.float16)
        parf[:fhs[l], l] = b.reshape(-1)
    in_maps = []
    for c in range(N_CORES):
        blk = adj[c * NL:(c + 1) * NL, :].astype(ml_dtypes.float8_e4m3)
        # [NL, N] -> (m, h, q, kp, i, p) -> [p, m, kp, i, h, q]
        adjt = blk.reshape(NT, 2, 64, KP, 2, 128).transpose(5, 0, 3, 4, 1, 2)
        m = {
            "adjt": np.ascontiguousarray(adjt),
            "xt_local": np.ascontiguousarray(
                x[c * NL:(c + 1) * NL, :].T.astype(np.float16)),
            "par16": par16,
            "parf": parf,
        }
        in_maps.append(m)
    return in_maps


def kernel(x, adj, W0, b0, aW0, ab0, W1, b1, aW1, ab1, W2, b2, aW2, ab2):
    inputs = dict(x=x, adj=adj, W0=W0, b0=b0, aW0=aW0, ab0=ab0,
                  W1=W1, b1=b1, aW1=aW1, ab1=ab1, W2=W2, b2=b2, aW2=aW2, ab2=ab2)
    ab = [float(np.asarray(inputs[f"ab{l}"]).reshape(-1)[0]) for l in range(3)]
    nc = _get_program(ab)
    in_maps = _make_in_maps(inputs)
    res = run_bass_kernel_spmd(nc, in_maps, list(range(N_CORES)))
    out = np.concatenate([res.results[c]["out"] for c in range(N_CORES)], axis=0)
    return out.astype(np.float32)


# revision 28
# speedup vs baseline: 7.7333x; 7.7333x over previous
"""GAT-style 3-layer attention graph network on 8 TRN2 NeuronCores.

Math: per layer, alpha[i,j] = adj[i,j]*exp(el[i]+er[j]+ab) / sum_k adj[i,k]*exp(el[i]+er[k]+ab)
The exp(el[i]) factor cancels between numerator and denominator, so with
w[j] = exp(er[j]+ab):
    out[i] = relu( (sum_j adj[i,j]*w[j]*h[j]) / (sum_j adj[i,j]*w[j]) )
i.e. one [N,N]@[N,F+1] matmul per layer against G = [h*w | w], with adj
constant across layers.

Distribution: row-shard adj across the 8 cores (1024 dest rows each). adj is
0/1 so it is exactly representable in fp8_e4m3. Layers 0 and 1 run the
aggregation as fp8 DoubleRow matmuls (G stored fp8_e4m3, 2 contraction rows
per PE pass -> 2x): lhsT is [128, 2, 64] (256-row contraction, 64 dest rows),
rhs [128, 2, F+1]. Layer 2 keeps G in fp16 (fp8 G there pushes rel err past
the gate) using the same adj tile via a strided [2, 64] lhsT AP.
The host pre-transposes each core's adj row-block into a single layout
[p, m, kp, i, h, q] fp8 that serves both paths; it stays SBUF-resident
(8MB/core) across all 3 layers.
Each layer all-gathers the 8192x(F+1) G matrix (fp8 for layers 0/1, fp16 for
layer 2) in two node-halves so the first gather overlaps the previous
aggregation; G blocks are staged to DRAM per 128-node block as soon as each
is built so only the collective + SBUF reload sit on the layer boundary.
"""
import numpy as np

import concourse.bass as bass
import concourse.mybir as mybir
import concourse.tile as tile
from concourse.masks import make_identity
from concourse.tile_rust import add_dep_helper
from concourse.bass_utils import run_bass_kernel_spmd

F32 = mybir.dt.float32
F16 = mybir.dt.float16
F8 = mybir.dt.float8e4    # adj storage + layers 0/1 G: e4m3

N_CORES = 8
N = 8192
NL = N // N_CORES          # 1024 local dest rows per core
NT = NL // 128             # 8 local node tiles
KT = N // 128              # 64 contraction tiles (fp16 path)
KP = KT // 2               # 32 contraction pair-tiles (DoubleRow path)
LEAK = 0.2
H1 = 4                     # node-blocks in the first gather half (of NT=8)
SPLITS = [4, 4, 4]         # per-layer first-gather-half block count
LADDER_SP = True          # run the half-1 gather ladder on SP too (A/B)
DR = mybir.MatmulPerfMode.DoubleRow


def _split_excess_waits(nc, max_waits=1):
    """This walrus build allows only one sync-wait command per instruction;
    split any instruction carrying more into preceding single-wait nops."""
    n_split = 0
    for fn in nc.m.functions:
        for bb in fn.blocks:
            insts = bb.instructions
            i = 0
            while i < len(insts):
                inst = insts[i]
                si = inst.sync_info
                if si is not None and len(si.on_wait) > max_waits:
                    waits = list(si.on_wait)
                    extra, keep = waits[:-max_waits], waits[-max_waits:]
                    nops = []
                    for j, w in enumerate(extra):
                        nop = mybir.InstNoOp(
                            name=f"{inst.name}-waitsplit-{j}", ins=[], outs=[]
                        )
                        nop.engine = inst.engine
                        nop.sync_info = mybir.SyncInfo(on_wait=[w], on_update=[])
                        nops.append(nop)
                    inst.sync_info = mybir.SyncInfo(
                        on_wait=keep, on_update=list(si.on_update)
                    )
                    insts[i:i] = nops
                    i += len(nops)
                    n_split += 1
                i += 1
    return n_split


def _build_program(ab, for_sim=False):
    """ab: the three attention bias floats (baked in as memset constants)."""
    fhs = [128, 128, 64]   # per-layer linear output width
    gdt = [F8, F8, F16]    # per-layer G storage dtype

    nc = bass.Bass(num_devices=N_CORES)

    # adj, host pre-tiled: adjt[p, m, kp, i, h, q] =
    #   adj_local[m*128 + h*64 + q, kp*256 + i*128 + p]
    adj_ext = nc.dram_tensor("adjt", [128, NT, KP, 2, 2, 64], F8,
                             kind="ExternalInput")
    x_ext = nc.dram_tensor("xt_local", [128, NL], F16, kind="ExternalInput")
    # fp16 params: cols [0:128)=W0T [128:256)=W1T [256:320)=W2T, 320+l=awr_l
    par16_ext = nc.dram_tensor("par16", [128, 324], F16, kind="ExternalInput")
    # fp32 params: col l = b_l (rows past fh zero-padded)
    parf_ext = nc.dram_tensor("parf", [128, 4], F32, kind="ExternalInput")
    out_ext = nc.dram_tensor("out", [NL, 64], F32, kind="ExternalOutput")

    # all-gather payload, split in two node-halves per layer: half h of layer
    # l holds rank blocks [128, 4*(fh+1)] with (p, t, f) = G[c*1024+(b0+t)*128+p, f]
    hblk = [[SPLITS[l], NT - SPLITS[l]] for l in range(3)]
    ag_ext = [[nc.dram_tensor(f"ag{l}h{h}",
                              [N_CORES * 128, hblk[l][h] * (fhs[l] + 1)],
                              gdt[l], addr_space="Shared") for h in range(2)]
              for l in range(3)]

    with tile.TileContext(nc) as tc:
        with (
            tc.tile_pool(name="const", bufs=1) as cp,
            tc.tile_pool(name="adjt", bufs=1) as ap_,
            tc.tile_pool(name="slabs", bufs=1) as sp,
            tc.tile_pool(name="gsb", bufs=2) as gp,
            tc.tile_pool(name="misc", bufs=2) as mp,
            tc.tile_pool(name="gloc", bufs=2) as glp,
            tc.tile_pool(name="dram", bufs=3, space="DRAM") as dp,
            tc.tile_pool(name="ptf", bufs=2, space="PSUM") as ptf,
            tc.tile_pool(name="plin", bufs=1, space="PSUM") as plin,
            tc.tile_pool(name="per", bufs=1, space="PSUM") as per,
            tc.tile_pool(name="pbig", bufs=4, space="PSUM") as pbig,
        ):
            # ---- constants / params ----
            ident16 = cp.tile([128, 128], F16)
            make_identity(nc, ident16[:])
            # load order tuned for the layer-0 prologue: W0/awr columns and
            # the first x slab first, everything else after
            par16 = cp.tile([128, 324], F16)
            parf = cp.tile([128, 4], F32)
            curT = sp.tile([128, NL], F16, tag="slab")
            nc.sync.dma_start(out=par16[:, 0:128], in_=par16_ext[:, 0:128])
            nc.sync.dma_start(out=curT[:, 0:256], in_=x_ext[:, 0:256])
            nc.sync.dma_start(out=par16[:, 128:324], in_=par16_ext[:, 128:324])
            nc.sync.dma_start(out=parf[:], in_=parf_ext.ap())
            nc.sync.dma_start(out=curT[:, 256:NL], in_=x_ext[:, 256:NL])
            woff = [0, 128, 256]
            wt_sb = [par16[:, woff[l]:woff[l] + fhs[l]] for l in range(3)]
            awr_sb = [par16[0:fhs[l], 320 + l:321 + l] for l in range(3)]
            b_sb = [parf[0:fhs[l], l:l + 1] for l in range(3)]
            ab_sb = []
            for l in range(3):
                t = cp.tile([128, 1], F32, tag=f"ab{l}")
                nc.gpsimd.memset(t[:], float(ab[l]))
                ab_sb.append(t)

            # ---- adj streams in per node-block chunk; layer-0 m-chains
            # consume chunks as they land. Piece size tuned so the transfer
            # time (~1us) matches the SWDGE generation cadence (~1us): the
            # DMA FIFO stays drained and G-store/gather/reload hops slot in
            # within ~1us instead of queueing behind a backlog ----
            adjT = ap_.tile([128, NT, KP, 2, 2, 64], F8)
            adj_insts = []
            for d in range(NT):
                for k0, k1 in ((0, 11), (11, 22), (22, 32)):
                    adj_insts.append((d, nc.gpsimd.dma_start(
                        out=adjT[:, d, k0:k1, :, :, :],
                        in_=adj_ext[:, d, k0:k1, :, :, :])))

            # local G staging in DRAM for the all-gather, written per block
            gld = [[dp.tile([128, hblk[l][h] * (fhs[l] + 1)], gdt[l],
                            tag="gld", name=f"gld{l}h{h}")
                    for h in range(2)] for l in range(3)]


            # ---- G-prep helper: one 128-node block of layer l's G ----
            # src_col: [128(fi), 128] fp16 column of transposed prev activations
            def prep_block(l, src_col, gl, m):
                fh = fhs[l]
                pl = plin.tile([128, 256], F32, tag="lin")
                nc.tensor.matmul(pl[0:fh, 0:128], wt_sb[l], src_col,
                                 start=True, stop=True)
                hcol = mp.tile([128, 128], F16, tag="hcol")
                nc.scalar.activation(
                    hcol[0:fh, :], pl[0:fh, 0:128],
                    mybir.ActivationFunctionType.Prelu,
                    bias=b_sb[l], scale=1.0, alpha=LEAK,
                )
                pe_ = per.tile([128, 2], F32, tag="er")
                nc.tensor.matmul(pe_[:, 0:1], hcol[0:fh, :], awr_sb[l],
                                 start=True, stop=True)
                ec = mp.tile([128, 1], F32, tag="expc")
                nc.scalar.activation(
                    ec[:], pe_[:, 0:1], mybir.ActivationFunctionType.Exp,
                    bias=ab_sb[l][:], scale=1.0,
                )
                ptg = ptf.tile([128, 128], F16, tag="ptf")
                nc.tensor.transpose(ptg[:, 0:fh], hcol[0:fh, :],
                                    ident16[0:fh, 0:fh])
                nc.vector.tensor_scalar_mul(gl[:, m, 0:fh], ptg[:, 0:fh], ec[:])
                nc.vector.tensor_copy(gl[:, m, fh:fh + 1], ec[:])

            gsb_tiles = {}

            def fire_gather(l, gl, h):
                """DRAM-stage node-half h of layer l's local G, all-gather
                it, then queue the SBUF reload of that half. One store per
                half (each HWDGE dma costs ~630ns of serial descriptor-gen).
                The two halves get separate SBUF tiles (tile-level deps stay
                independent) and separate DGE queues (SP for half 0, Act for
                half 1) so their store/collective/reload ladders overlap
                instead of serializing on one in-order queue."""
                fh = fhs[l]
                b0 = 0 if h == 0 else SPLITS[l]
                nb = hblk[l][h]
                eng = nc.sync if (h == 0 or LADDER_SP) else nc.scalar
                eng.dma_start(out=gld[l][h][:], in_=gl[:, b0:b0 + nb, :])
                if for_sim:
                    # stand-in approximating the collective's wire time
                    # (~7/8 of the half moves over D2D); the real collective
                    # runs on the CC engine and does not consume HWDGE slots
                    eng.dma_start(
                        out=ag_ext[l][h][0:128, :], in_=gld[l][h][:])
                else:
                    nc.gpsimd.collective_compute(
                        "AllGather", mybir.AluOpType.bypass,
                        replica_groups=[list(range(N_CORES))],
                        ins=[gld[l][h].opt()], outs=[ag_ext[l][h].ap().opt()],
                    )
                if h == 0:
                    # one SBUF tile per half ([c, t, f] within the half:
                    # 4*129B contiguous per (p, c) keeps DMA descriptors
                    # >=512B) so the two reloads stay independent deps
                    gsb_tiles[l] = (
                        gp.tile([128, N_CORES, SPLITS[l], fhs[l] + 1],
                                gdt[l], tag="gsbA", name="gsba"),
                        gp.tile([128, N_CORES, NT - SPLITS[l], fhs[l] + 1],
                                gdt[l], tag="gsbB", name="gsbb"),
                    )
                return eng.dma_start(
                    out=gsb_tiles[l][h][:, :, :, :],
                    in_=ag_ext[l][h].ap().rearrange(
                        "(c p) (t f) -> p c t f", p=128, f=fh + 1
                    ),
                )

            # ---- layer 0 G from x (overlaps the adj load): pipelined over
            # the four 256-node slabs (all linears issue back-to-back on the
            # then-idle pbig banks, so only two Act round-trips sit on the
            # startup critical path) ----
            gl_cur = glp.tile([128, NT, fhs[0] + 1], F8, tag="gloc")
            fh0 = fhs[0]
            pls = []
            for j in range(4):
                plx = pbig.tile([128, 256], F32, tag="big", name="plx")
                nc.tensor.matmul(plx[0:fh0, 0:256], wt_sb[0],
                                 curT[:, 256 * j:256 * (j + 1)],
                                 start=True, stop=True)
                pls.append(plx)
            hcs = []
            for j in range(4):
                hc = mp.tile([128, 256], F16, tag="hcol2", bufs=4, name="hc")
                if j % 2 == 0:
                    nc.scalar.activation(
                        hc[0:fh0, :], pls[j][0:fh0, 0:256],
                        mybir.ActivationFunctionType.Prelu,
                        bias=b_sb[0], scale=1.0, alpha=LEAK,
                    )
                else:
                    # leaky-relu on DVE so the four prologue activations
                    # split across two engines
                    zt = mp.tile([128, 256], F16, tag="zt", bufs=2, name="zt")
                    nc.vector.tensor_scalar_add(
                        zt[0:fh0, :], pls[j][0:fh0, 0:256], b_sb[0])
                    nc.vector.scalar_tensor_tensor(
                        hc[0:fh0, :], zt[0:fh0, :], LEAK, zt[0:fh0, :],
                        op0=mybir.AluOpType.mult, op1=mybir.AluOpType.max)
                hcs.append(hc)
            ecs = []
            for j in range(4):
                pe_ = per.tile([128, 2], F32, tag="er")
                for i in range(2):
                    nc.tensor.matmul(pe_[:, i:i + 1],
                                     hcs[j][0:fh0, 128 * i:128 * (i + 1)],
                                     awr_sb[0], start=True, stop=True)
                ec = mp.tile([128, 2], F32, tag="expc2", bufs=4, name="ec")
                nc.scalar.activation(
                    ec[:], pe_[:, 0:2], mybir.ActivationFunctionType.Exp,
                    bias=ab_sb[0][:], scale=1.0,
                )
                ecs.append(ec)
            for j in range(4):
                for i in range(2):
                    m = 2 * j + i
                    ptg = ptf.tile([128, 128], F16, tag="ptf")
                    nc.tensor.transpose(ptg[:, 0:fh0],
                                        hcs[j][0:fh0, 128 * i:128 * (i + 1)],
                                        ident16[0:fh0, 0:fh0])
                    nc.vector.tensor_scalar_mul(
                        gl_cur[:, m, 0:fh0], ptg[:, 0:fh0], ecs[j][:, i:i + 1])
                    nc.vector.tensor_copy(
                        gl_cur[:, m, fh0:fh0 + 1], ecs[j][:, i:i + 1])
                if j == 1:
                    g0h0 = fire_gather(0, gl_cur, 0)
            g0h1 = fire_gather(0, gl_cur, 1)
            # adj blocks 4-7 are not consumed until ~20us in; make them yield
            # the DMA FIFO to layer 0's G-gather ladder
            for d, inst in adj_insts:
                if d >= 4:
                    add_dep_helper(inst.ins, g0h0.ins, sync=True,
                                   reason="late adj yields to L0 G reload")
                    add_dep_helper(inst.ins, g0h1.ins, sync=True,
                                   reason="late adj yields to L0 G reload")

            # ---- layers ----
            def make_epilogue(l, gl_next, ostage):
                fh = fhs[l]

                def epilogue(m, bp):
                    if l < 2:
                        # DoubleRow output: two banks, each h-half in
                        # partitions 0:64 (one accumulation group per bank --
                        # two groups in one bank wedge the runtime)
                        recip = mp.tile([128, 2], F32, tag="recip")
                        for h in range(2):
                            nc.vector.reciprocal(recip[0:64, h:h + 1],
                                                 bp[h][0:64, fh:fh + 1])
                        h2 = mp.tile([64, 256], F16, tag="h2")
                        for h in range(2):
                            # relu(num * recip) on DVE: the Act engine is the
                            # steady-state bottleneck otherwise
                            nc.vector.tensor_scalar(
                                h2[:, 128 * h:128 * (h + 1)],
                                bp[h][0:64, 0:fh],
                                recip[0:64, h:h + 1], 0.0,
                                op0=mybir.AluOpType.mult,
                                op1=mybir.AluOpType.max)
                        pt = ptf.tile([128, 128], F16, tag="ptf")
                        for h in range(2):
                            nc.tensor.transpose(
                                pt[:, 64 * h:64 * (h + 1)],
                                h2[0:64, 128 * h:128 * (h + 1)],
                                ident16[0:64, 0:64])
                        cpcol = mp.tile([128, 128], F16, tag="cpcol")
                        nc.vector.tensor_copy(cpcol[:], pt[:, 0:128])
                        prep_block(l + 1, cpcol[:], gl_next, m)
                        if m == SPLITS[l + 1] - 1:
                            fire_gather(l + 1, gl_next, 0)
                        elif m == NT - 1:
                            fire_gather(l + 1, gl_next, 1)
                    else:
                        recip = mp.tile([128, 2], F32, tag="recip")
                        nc.vector.reciprocal(recip[:, 0:1], bp[:, fh:fh + 1])
                        ot = mp.tile([128, 64], F32, tag="ot")
                        nc.scalar.activation(
                            ot[:], bp[:, 0:fh],
                            mybir.ActivationFunctionType.Relu,
                            bias=0.0, scale=recip[:, 0:1],
                        )
                        nc.sync.dma_start(
                            out=out_ext[m * 128:(m + 1) * 128, :], in_=ot[:])

                return epilogue

            from collections import deque
            pending = deque()

            def flush(n=None):
                k = len(pending) if n is None else min(n, len(pending))
                for _ in range(k):
                    f, pm, pbp = pending.popleft()
                    f(pm, pbp)

            for l in range(3):
                fh = fhs[l]
                gsb = gsb_tiles[l]
                if l < 2:
                    gl_next = glp.tile([128, NT, fhs[l + 1] + 1], gdt[l + 1],
                                       tag="gloc")
                    ostage = None
                else:
                    gl_next = None
                    ostage = True
                epi = make_epilogue(l, gl_next, ostage)

                if l < 2:
                    # DoubleRow items (h, kp); gather-half 0 covers
                    # t0 < SPLITS[l] i.e. kp % 4 < SPLITS[l] // 2
                    sl2 = SPLITS[l] // 2
                    items = [(h, kp) for h in range(2)
                             for kp in range(KP) if kp % 4 < sl2] + \
                            [(h, kp) for h in range(2)
                             for kp in range(KP) if kp % 4 >= sl2]
                    first_i = {}
                    last_i = {}
                    for i, (h, kp) in enumerate(items):
                        first_i.setdefault(h, i)
                        last_i[h] = i
                    n1 = 2 * 8 * sl2  # first-half instruction count

                    def mm_run(bp, m, i0, i1):
                        for i in range(i0, i1):
                            h, kp = items[i]
                            t0 = 2 * (kp % 4)
                            gt, tb = (gsb[0], t0) if t0 < SPLITS[l] else \
                                     (gsb[1], t0 - SPLITS[l])
                            nc.tensor.matmul(
                                bp[h][0:64, 0:fh + 1],
                                adjT[:, m, kp, :, h, :],
                                gt[:, kp // 4, tb:tb + 2, :],
                                start=(i == first_i[h]), stop=(i == last_i[h]),
                                perf_mode=DR,
                            )
                else:
                    # fp16 path; gather-half 0 covers t = k % 8 < 4
                    ks = [k for k in range(KT) if k % NT < SPLITS[l]] + \
                         [k for k in range(KT) if k % NT >= SPLITS[l]]
                    n1 = KT // 2

                    def mm_run(bp, m, i0, i1):
                        for i in range(i0, i1):
                            k = ks[i]
                            t = k % NT
                            gt, tb = (gsb[0], t) if t < SPLITS[l] else \
                                     (gsb[1], t - SPLITS[l])
                            nc.tensor.matmul(
                                bp[:, 0:fh + 1],
                                adjT[:, m, k // 2, k % 2, :, :],
                                gt[:, k // NT, tb, :],
                                start=(i == 0), stop=(i == KT - 1),
                            )

                nk = KT  # total MM instructions per m-chain (both paths)

                def alloc_bp():
                    # one accumulation group per PSUM bank: DoubleRow chains
                    # need a bank pair (the two 64-row h-groups)
                    if l < 2:
                        return (pbig.tile([128, fh + 1], F32, tag="big", name="bpa"),
                                pbig.tile([128, fh + 1], F32, tag="big", name="bpb"))
                    return pbig.tile([128, fh + 1], F32, tag="big", name="bpc")

                # each pending epilogue is flushed right before its banks are
                # reallocated (2 banks/chain for l<2 -> 2-chain pipeline);
                # seam: the previous layer's m=7 epilogue (which fires the
                # G-half-2 gather) lands before any second-half MM
                flush(1)
                bp0 = alloc_bp()
                mm_run(bp0, 0, 0, n1)
                flush()
                bp1 = alloc_bp()
                mm_run(bp1, 1, 0, n1)
                mm_run(bp0, 0, n1, nk)
                mm_run(bp1, 1, n1, nk)
                pending.append((epi, 0, bp0))
                pending.append((epi, 1, bp1))
                for m in range(2, NT):
                    flush(1)
                    bp = alloc_bp()
                    mm_run(bp, m, 0, n1)
                    mm_run(bp, m, n1, nk)
                    pending.append((epi, m, bp))
            flush()

    _split_excess_waits(nc)
    return nc


_PROG_CACHE = {}


def _get_program(ab):
    key = tuple(round(a, 9) for a in ab)
    if key not in _PROG_CACHE:
        _PROG_CACHE[key] = _build_program(ab)
    return _PROG_CACHE[key]


def _make_in_maps(inputs):
    """Build the per-core input maps from the full (unsharded) input dict."""
    import ml_dtypes
    fhs = [128, 128, 64]
    x = np.asarray(inputs["x"], np.float32)
    adj = np.asarray(inputs["adj"], np.float32)

    par16 = np.zeros((128, 324), np.float16)
    parf = np.zeros((128, 4), np.float32)
    woff = [0, 128, 256]
    for l in range(3):
        W = np.asarray(inputs[f"W{l}"], np.float32)
        b = np.asarray(inputs[f"b{l}"], np.float32)
        aW = np.asarray(inputs[f"aW{l}"], np.float32)
        par16[:, woff[l]:woff[l] + fhs[l]] = W.T.astype(np.float16)
        par16[:fhs[l], 320 + l] = aW[0, fhs[l]:2 * fhs[l]].astype(np.float16)
        parf[:fhs[l], l] = b.reshape(-1)
    in_maps = []
    for c in range(N_CORES):
        blk = adj[c * NL:(c + 1) * NL, :].astype(ml_dtypes.float8_e4m3)
        # [NL, N] -> (m, h, q, kp, i, p) -> [p, m, kp, i, h, q]
        adjt = blk.reshape(NT, 2, 64, KP, 2, 128).transpose(5, 0, 3, 4, 1, 2)
        m = {
            "adjt": np.ascontiguousarray(adjt),
            "xt_local": np.ascontiguousarray(
                x[c * NL:(c + 1) * NL, :].T.astype(np.float16)),
            "par16": par16,
            "parf": parf,
        }
        in_maps.append(m)
    return in_maps


def kernel(x, adj, W0, b0, aW0, ab0, W1, b1, aW1, ab1, W2, b2, aW2, ab2):
    inputs = dict(x=x, adj=adj, W0=W0, b0=b0, aW0=aW0, ab0=ab0,
                  W1=W1, b1=b1, aW1=aW1, ab1=ab1, W2=W2, b2=b2, aW2=aW2, ab2=ab2)
    ab = [float(np.asarray(inputs[f"ab{l}"]).reshape(-1)[0]) for l in range(3)]
    nc = _get_program(ab)
    in_maps = _make_in_maps(inputs)
    res = run_bass_kernel_spmd(nc, in_maps, list(range(N_CORES)))
    out = np.concatenate([res.results[c]["out"] for c in range(N_CORES)], axis=0)
    return out.astype(np.float32)


# revision 41
# speedup vs baseline: 7.8501x; 1.0151x over previous
"""GAT-style 3-layer attention graph network on 8 TRN2 NeuronCores.

Math: per layer, alpha[i,j] = adj[i,j]*exp(el[i]+er[j]+ab) / sum_k adj[i,k]*exp(el[i]+er[k]+ab)
The exp(el[i]) factor cancels between numerator and denominator, so with
w[j] = exp(er[j]+ab):
    out[i] = relu( (sum_j adj[i,j]*w[j]*h[j]) / (sum_j adj[i,j]*w[j]) )
i.e. one [N,N]@[N,F+1] matmul per layer against G = [h*w | w], with adj
constant across layers.

Distribution: row-shard adj across the 8 cores (1024 dest rows each). adj is
0/1 so it is exactly representable in fp8_e4m3. Layers 0 and 1 run the
aggregation as fp8 DoubleRow matmuls (G stored fp8_e4m3, 2 contraction rows
per PE pass -> 2x): lhsT is [128, 2, 64] (256-row contraction, 64 dest rows),
rhs [128, 2, F+1]. Layer 2 keeps G in fp16 (fp8 G there pushes rel err past
the gate) using the same adj tile via a strided [2, 64] lhsT AP.
The host pre-transposes each core's adj row-block into a single layout
[p, m, kp, i, h, q] fp8 that serves both paths; it stays SBUF-resident
(8MB/core) across all 3 layers.
Each layer all-gathers the 8192x(F+1) G matrix (fp8 for layers 0/1, fp16 for
layer 2) in two node-halves so the first gather (fired after half the
aggregation chains) overlaps the rest of the layer, and only the second
half's store/collective/reload ladder sits on the layer boundary.
"""
import numpy as np

import concourse.bass as bass
import concourse.mybir as mybir
import concourse.tile as tile
from concourse.masks import make_identity
from concourse.tile_rust import add_dep_helper
from concourse.bass_utils import run_bass_kernel_spmd

F32 = mybir.dt.float32
F16 = mybir.dt.float16
F8 = mybir.dt.float8e4    # adj storage + layers 0/1 G: e4m3

N_CORES = 8
N = 8192
NL = N // N_CORES          # 1024 local dest rows per core
NT = NL // 128             # 8 local node tiles
KT = N // 128              # 64 contraction tiles (fp16 path)
KP = KT // 2               # 32 contraction pair-tiles (DoubleRow path)
LEAK = 0.2
H1 = 4                     # node-blocks in the first gather half (of NT=8)
SPLITS = [4, 4, 4]         # per-layer first-gather-half block count
LADDER_SP = True          # run the half-1 gather ladder on SP too (A/B)
DR = mybir.MatmulPerfMode.DoubleRow


def _split_excess_waits(nc, max_waits=1):
    """This walrus build allows only one sync-wait command per instruction;
    split any instruction carrying more into preceding single-wait nops."""
    n_split = 0
    for fn in nc.m.functions:
        for bb in fn.blocks:
            insts = bb.instructions
            i = 0
            while i < len(insts):
                inst = insts[i]
                si = inst.sync_info
                if si is not None and len(si.on_wait) > max_waits:
                    waits = list(si.on_wait)
                    extra, keep = waits[:-max_waits], waits[-max_waits:]
                    nops = []
                    for j, w in enumerate(extra):
                        nop = mybir.InstNoOp(
                            name=f"{inst.name}-waitsplit-{j}", ins=[], outs=[]
                        )
                        nop.engine = inst.engine
                        nop.sync_info = mybir.SyncInfo(on_wait=[w], on_update=[])
                        nops.append(nop)
                    inst.sync_info = mybir.SyncInfo(
                        on_wait=keep, on_update=list(si.on_update)
                    )
                    insts[i:i] = nops
                    i += len(nops)
                    n_split += 1
                i += 1
    return n_split


def _build_program(ab, for_sim=False):
    """ab: the three attention bias floats (baked in as memset constants)."""
    fhs = [128, 128, 64]   # per-layer linear output width
    gdt = [F8, F8, F16]    # per-layer G storage dtype

    nc = bass.Bass(num_devices=N_CORES)

    # adj, host pre-tiled: adjt[p, m, kp, i, h, q] =
    #   adj_local[m*128 + h*64 + q, kp*256 + i*128 + p]
    adj_ext = nc.dram_tensor("adjt", [128, NT, KP, 2, 2, 64], F8,
                             kind="ExternalInput")
    x_ext = nc.dram_tensor("xt_local", [128, NL], F16, kind="ExternalInput")
    # fp16 params: cols [0:128)=W0T [128:256)=W1T [256:320)=W2T, 320+l=awr_l
    par16_ext = nc.dram_tensor("par16", [128, 324], F16, kind="ExternalInput")
    # fp32 params: col l = b_l (rows past fh zero-padded)
    parf_ext = nc.dram_tensor("parf", [128, 4], F32, kind="ExternalInput")
    out_ext = nc.dram_tensor("out", [NL, 64], F32, kind="ExternalOutput")

    # all-gather payload, split in two node-halves per layer: half h of layer
    # l holds rank blocks [128, 4*(fh+1)] with (p, t, f) = G[c*1024+(b0+t)*128+p, f]
    hblk = [[SPLITS[l], NT - SPLITS[l]] for l in range(3)]
    ag_ext = [[nc.dram_tensor(f"ag{l}h{h}",
                              [N_CORES * 128, hblk[l][h] * (fhs[l] + 1)],
                              gdt[l], addr_space="Shared") for h in range(2)]
              for l in range(3)]

    with tile.TileContext(nc) as tc:
        with (
            tc.tile_pool(name="const", bufs=1) as cp,
            tc.tile_pool(name="adjt", bufs=1) as ap_,
            tc.tile_pool(name="slabs", bufs=1) as sp,
            tc.tile_pool(name="gsb", bufs=2) as gp,
            tc.tile_pool(name="misc", bufs=2) as mp,
            tc.tile_pool(name="gloc", bufs=2) as glp,
            tc.tile_pool(name="dram", bufs=3, space="DRAM") as dp,
            tc.tile_pool(name="ptf", bufs=2, space="PSUM") as ptf,
            tc.tile_pool(name="plin", bufs=1, space="PSUM") as plin,
            tc.tile_pool(name="per", bufs=1, space="PSUM") as per,
            tc.tile_pool(name="pbig", bufs=4, space="PSUM") as pbig,
        ):
            # ---- constants / params ----
            ident16 = cp.tile([128, 128], F16)
            make_identity(nc, ident16[:])
            # load order tuned for the layer-0 prologue: W0/awr columns and
            # the first x slab first, everything else after
            par16 = cp.tile([128, 324], F16)
            parf = cp.tile([128, 4], F32)
            curT = sp.tile([128, NL], F16, tag="slab")
            nc.sync.dma_start(out=par16[:, 0:128], in_=par16_ext[:, 0:128])
            nc.sync.dma_start(out=curT[:, 0:256], in_=x_ext[:, 0:256])
            nc.sync.dma_start(out=curT[:, 256:NL], in_=x_ext[:, 256:NL])
            nc.sync.dma_start(out=par16[:, 128:324], in_=par16_ext[:, 128:324])
            nc.sync.dma_start(out=parf[:], in_=parf_ext.ap())
            woff = [0, 128, 256]
            wt_sb = [par16[:, woff[l]:woff[l] + fhs[l]] for l in range(3)]
            awr_sb = [par16[0:fhs[l], 320 + l:321 + l] for l in range(3)]
            b_sb = [parf[0:fhs[l], l:l + 1] for l in range(3)]
            ab_sb = []
            for l in range(3):
                t = cp.tile([128, 1], F32, tag=f"ab{l}")
                nc.gpsimd.memset(t[:], float(ab[l]))
                ab_sb.append(t)

            # ---- adj streams in per node-block chunk; layer-0 m-chains
            # consume chunks as they land. Piece size tuned so the transfer
            # time (~1us) matches the SWDGE generation cadence (~1us): the
            # DMA FIFO stays drained and G-store/gather/reload hops slot in
            # within ~1us instead of queueing behind a backlog ----
            adjT = ap_.tile([128, NT, KP, 2, 2, 64], F8)
            adj_insts = []
            for d in range(NT):
                for k0, k1 in ((0, 11), (11, 22), (22, 32)):
                    adj_insts.append((d, nc.gpsimd.dma_start(
                        out=adjT[:, d, k0:k1, :, :, :],
                        in_=adj_ext[:, d, k0:k1, :, :, :])))

            # local G staging in DRAM for the all-gather, written per block
            gld = [[dp.tile([128, hblk[l][h] * (fhs[l] + 1)], gdt[l],
                            tag="gld", name=f"gld{l}h{h}")
                    for h in range(2)] for l in range(3)]


            # ---- G-prep helper: one 128-node block of layer l's G ----
            # src_col: [128(fi), 128] fp16 column of transposed prev activations
            def prep_block(l, src_col, gl, m):
                fh = fhs[l]
                pl = plin.tile([128, 256], F32, tag="lin")
                nc.tensor.matmul(pl[0:fh, 0:128], wt_sb[l], src_col,
                                 start=True, stop=True)
                hcol = mp.tile([128, 128], F16, tag="hcol")
                nc.scalar.activation(
                    hcol[0:fh, :], pl[0:fh, 0:128],
                    mybir.ActivationFunctionType.Prelu,
                    bias=b_sb[l], scale=1.0, alpha=LEAK,
                )
                pe_ = per.tile([128, 2], F32, tag="er")
                nc.tensor.matmul(pe_[:, 0:1], hcol[0:fh, :], awr_sb[l],
                                 start=True, stop=True)
                ec = mp.tile([128, 1], F32, tag="expc")
                nc.scalar.activation(
                    ec[:], pe_[:, 0:1], mybir.ActivationFunctionType.Exp,
                    bias=ab_sb[l][:], scale=1.0,
                )
                ptg = ptf.tile([128, 128], F16, tag="ptf")
                nc.tensor.transpose(ptg[:, 0:fh], hcol[0:fh, :],
                                    ident16[0:fh, 0:fh])
                nc.vector.tensor_scalar_mul(gl[:, m, 0:fh], ptg[:, 0:fh], ec[:])
                nc.vector.tensor_copy(gl[:, m, fh:fh + 1], ec[:])

            gsb_tiles = {}

            def fire_gather(l, gl, h):
                """DRAM-stage node-half h of layer l's local G, all-gather
                it, then queue the SBUF reload of that half. One store per
                half (each HWDGE dma costs ~630ns of serial descriptor-gen).
                The two halves get separate SBUF tiles (tile-level deps stay
                independent) and separate DGE queues (SP for half 0, Act for
                half 1) so their store/collective/reload ladders overlap
                instead of serializing on one in-order queue."""
                fh = fhs[l]
                b0 = 0 if h == 0 else SPLITS[l]
                nb = hblk[l][h]
                eng = nc.sync if (h == 0 or LADDER_SP) else nc.scalar
                eng.dma_start(out=gld[l][h][:], in_=gl[:, b0:b0 + nb, :])
                if for_sim:
                    # stand-in approximating the collective's wire time
                    # (~7/8 of the half moves over D2D); the real collective
                    # runs on the CC engine and does not consume HWDGE slots
                    eng.dma_start(
                        out=ag_ext[l][h][0:128, :], in_=gld[l][h][:])
                else:
                    nc.gpsimd.collective_compute(
                        "AllGather", mybir.AluOpType.bypass,
                        replica_groups=[list(range(N_CORES))],
                        ins=[gld[l][h].opt()], outs=[ag_ext[l][h].ap().opt()],
                    )
                if h == 0:
                    # one SBUF tile per half ([c, t, f] within the half:
                    # 4*129B contiguous per (p, c) keeps DMA descriptors
                    # >=512B) so the two reloads stay independent deps
                    gsb_tiles[l] = (
                        gp.tile([128, N_CORES, SPLITS[l], fhs[l] + 1],
                                gdt[l], tag="gsbA", name="gsba"),
                        gp.tile([128, N_CORES, NT - SPLITS[l], fhs[l] + 1],
                                gdt[l], tag="gsbB", name="gsbb"),
                    )
                return eng.dma_start(
                    out=gsb_tiles[l][h][:, :, :, :],
                    in_=ag_ext[l][h].ap().rearrange(
                        "(c p) (t f) -> p c t f", p=128, f=fh + 1
                    ),
                )

            # ---- layer 0 G from x (overlaps the adj load): pipelined over
            # the four 256-node slabs (all linears issue back-to-back on the
            # then-idle pbig banks, so only two Act round-trips sit on the
            # startup critical path) ----
            gl_cur = glp.tile([128, NT, fhs[0] + 1], F8, tag="gloc")
            fh0 = fhs[0]
            pls = []
            for j in range(4):
                plx = pbig.tile([128, 256], F32, tag="big", name="plx")
                nc.tensor.matmul(plx[0:fh0, 0:256], wt_sb[0],
                                 curT[:, 256 * j:256 * (j + 1)],
                                 start=True, stop=True)
                pls.append(plx)
            hcs = []
            for j in range(4):
                hc = mp.tile([128, 256], F16, tag="hcol2", bufs=4, name="hc")
                if j % 2 == 1:
                    nc.scalar.activation(
                        hc[0:fh0, :], pls[j][0:fh0, 0:256],
                        mybir.ActivationFunctionType.Prelu,
                        bias=b_sb[0], scale=1.0, alpha=LEAK,
                    )
                else:
                    # leaky-relu on DVE so the four prologue activations
                    # split across two engines
                    zt = mp.tile([128, 256], F16, tag="zt", bufs=2, name="zt")
                    nc.vector.tensor_scalar_add(
                        zt[0:fh0, :], pls[j][0:fh0, 0:256], b_sb[0])
                    nc.vector.scalar_tensor_tensor(
                        hc[0:fh0, :], zt[0:fh0, :], LEAK, zt[0:fh0, :],
                        op0=mybir.AluOpType.mult, op1=mybir.AluOpType.max)
                hcs.append(hc)
            ecs = []
            for j in range(4):
                pe_ = per.tile([128, 2], F32, tag="er")
                for i in range(2):
                    nc.tensor.matmul(pe_[:, i:i + 1],
                                     hcs[j][0:fh0, 128 * i:128 * (i + 1)],
                                     awr_sb[0], start=True, stop=True)
                ec = mp.tile([128, 2], F32, tag="expc2", bufs=4, name="ec")
                nc.scalar.activation(
                    ec[:], pe_[:, 0:2], mybir.ActivationFunctionType.Exp,
                    bias=ab_sb[0][:], scale=1.0,
                )
                ecs.append(ec)
            for j in range(4):
                for i in range(2):
                    m = 2 * j + i
                    ptg = ptf.tile([128, 128], F16, tag="ptf")
                    nc.tensor.transpose(ptg[:, 0:fh0],
                                        hcs[j][0:fh0, 128 * i:128 * (i + 1)],
                                        ident16[0:fh0, 0:fh0])
                    nc.vector.tensor_scalar_mul(
                        gl_cur[:, m, 0:fh0], ptg[:, 0:fh0], ecs[j][:, i:i + 1])
                    nc.vector.tensor_copy(
                        gl_cur[:, m, fh0:fh0 + 1], ecs[j][:, i:i + 1])
                if j == 1:
                    g0h0 = fire_gather(0, gl_cur, 0)
            g0h1 = fire_gather(0, gl_cur, 1)
            # adj blocks 4-7 are not consumed until ~20us in; make them yield
            # the DMA FIFO to layer 0's G-gather ladder
            for d, inst in adj_insts:
                if d >= 4:
                    add_dep_helper(inst.ins, g0h0.ins, sync=True,
                                   reason="late adj yields to L0 G reload")
                    add_dep_helper(inst.ins, g0h1.ins, sync=True,
                                   reason="late adj yields to L0 G reload")

            # ---- layers ----
            def make_epilogue(l, gl_next, ostage):
                fh = fhs[l]

                def epilogue(m, bp):
                    if l < 2:
                        # DoubleRow output: two banks, each h-half in
                        # partitions 0:64 (one accumulation group per bank --
                        # two groups in one bank wedge the runtime)
                        recip = mp.tile([128, 2], F32, tag="recip")
                        for h in range(2):
                            nc.vector.reciprocal(recip[0:64, h:h + 1],
                                                 bp[h][0:64, fh:fh + 1])
                        h2 = mp.tile([64, 256], F16, tag="h2")
                        for h in range(2):
                            # relu(num * recip) on DVE: the Act engine is the
                            # steady-state bottleneck otherwise
                            nc.vector.tensor_scalar(
                                h2[:, 128 * h:128 * (h + 1)],
                                bp[h][0:64, 0:fh],
                                recip[0:64, h:h + 1], 0.0,
                                op0=mybir.AluOpType.mult,
                                op1=mybir.AluOpType.max)
                        pt = ptf.tile([128, 128], F16, tag="ptf")
                        for h in range(2):
                            nc.tensor.transpose(
                                pt[:, 64 * h:64 * (h + 1)],
                                h2[0:64, 128 * h:128 * (h + 1)],
                                ident16[0:64, 0:64])
                        cpcol = mp.tile([128, 128], F16, tag="cpcol")
                        nc.vector.tensor_copy(cpcol[:], pt[:, 0:128])
                        prep_block(l + 1, cpcol[:], gl_next, m)
                        if m == SPLITS[l + 1] - 1:
                            fire_gather(l + 1, gl_next, 0)
                        elif m == NT - 1:
                            fire_gather(l + 1, gl_next, 1)
                    else:
                        recip = mp.tile([128, 2], F32, tag="recip")
                        nc.vector.reciprocal(recip[:, 0:1], bp[:, fh:fh + 1])
                        ot = mp.tile([128, 64], F32, tag="ot")
                        nc.scalar.activation(
                            ot[:], bp[:, 0:fh],
                            mybir.ActivationFunctionType.Relu,
                            bias=0.0, scale=recip[:, 0:1],
                        )
                        nc.sync.dma_start(
                            out=out_ext[m * 128:(m + 1) * 128, :], in_=ot[:])

                return epilogue

            from collections import deque
            pending = deque()

            def flush(n=None):
                k = len(pending) if n is None else min(n, len(pending))
                for _ in range(k):
                    f, pm, pbp = pending.popleft()
                    f(pm, pbp)

            for l in range(3):
                fh = fhs[l]
                gsb = gsb_tiles[l]
                if l < 2:
                    gl_next = glp.tile([128, NT, fhs[l + 1] + 1], gdt[l + 1],
                                       tag="gloc")
                    ostage = None
                else:
                    gl_next = None
                    ostage = True
                epi = make_epilogue(l, gl_next, ostage)

                if l < 2:
                    # DoubleRow items (h, kp); gather-half 0 covers
                    # t0 < SPLITS[l] i.e. kp % 4 < SPLITS[l] // 2
                    sl2 = SPLITS[l] // 2
                    items = [(h, kp) for h in range(2)
                             for kp in range(KP) if kp % 4 < sl2] + \
                            [(h, kp) for h in range(2)
                             for kp in range(KP) if kp % 4 >= sl2]
                    first_i = {}
                    last_i = {}
                    for i, (h, kp) in enumerate(items):
                        first_i.setdefault(h, i)
                        last_i[h] = i
                    n1 = 2 * 8 * sl2  # first-half instruction count

                    def mm_run(bp, m, i0, i1):
                        for i in range(i0, i1):
                            h, kp = items[i]
                            t0 = 2 * (kp % 4)
                            gt, tb = (gsb[0], t0) if t0 < SPLITS[l] else \
                                     (gsb[1], t0 - SPLITS[l])
                            nc.tensor.matmul(
                                bp[h][0:64, 0:fh + 1],
                                adjT[:, m, kp, :, h, :],
                                gt[:, kp // 4, tb:tb + 2, :],
                                start=(i == first_i[h]), stop=(i == last_i[h]),
                                perf_mode=DR,
                            )
                else:
                    # fp16 path; gather-half 0 covers t = k % 8 < 4
                    ks = [k for k in range(KT) if k % NT < SPLITS[l]] + \
                         [k for k in range(KT) if k % NT >= SPLITS[l]]
                    n1 = KT // 2

                    def mm_run(bp, m, i0, i1):
                        for i in range(i0, i1):
                            k = ks[i]
                            t = k % NT
                            gt, tb = (gsb[0], t) if t < SPLITS[l] else \
                                     (gsb[1], t - SPLITS[l])
                            nc.tensor.matmul(
                                bp[:, 0:fh + 1],
                                adjT[:, m, k // 2, k % 2, :, :],
                                gt[:, k // NT, tb, :],
                                start=(i == 0), stop=(i == KT - 1),
                            )

                nk = KT  # total MM instructions per m-chain (both paths)

                def alloc_bp():
                    # one accumulation group per PSUM bank: DoubleRow chains
                    # need a bank pair (the two 64-row h-groups)
                    if l < 2:
                        return (pbig.tile([128, fh + 1], F32, tag="big", name="bpa"),
                                pbig.tile([128, fh + 1], F32, tag="big", name="bpb"))
                    return pbig.tile([128, fh + 1], F32, tag="big", name="bpc")

                # each pending epilogue is flushed right before its banks are
                # reallocated (2 banks/chain for l<2 -> 2-chain pipeline);
                # seam: the previous layer's m=7 epilogue (which fires the
                # G-half-2 gather) lands before any second-half MM
                flush(1)
                bp0 = alloc_bp()
                mm_run(bp0, 0, 0, n1)
                flush()
                bp1 = alloc_bp()
                mm_run(bp1, 1, 0, n1)
                mm_run(bp0, 0, n1, nk)
                mm_run(bp1, 1, n1, nk)
                pending.append((epi, 0, bp0))
                pending.append((epi, 1, bp1))
                for m in range(2, NT):
                    flush(1)
                    bp = alloc_bp()
                    mm_run(bp, m, 0, n1)
                    mm_run(bp, m, n1, nk)
                    pending.append((epi, m, bp))
            flush()

    _split_excess_waits(nc)
    return nc


_PROG_CACHE = {}


def _get_program(ab):
    key = tuple(round(a, 9) for a in ab)
    if key not in _PROG_CACHE:
        _PROG_CACHE[key] = _build_program(ab)
    return _PROG_CACHE[key]


def _make_in_maps(inputs):
    """Build the per-core input maps from the full (unsharded) input dict."""
    import ml_dtypes
    fhs = [128, 128, 64]
    x = np.asarray(inputs["x"], np.float32)
    adj = np.asarray(inputs["adj"], np.float32)

    par16 = np.zeros((128, 324), np.float16)
    parf = np.zeros((128, 4), np.float32)
    woff = [0, 128, 256]
    for l in range(3):
        W = np.asarray(inputs[f"W{l}"], np.float32)
        b = np.asarray(inputs[f"b{l}"], np.float32)
        aW = np.asarray(inputs[f"aW{l}"], np.float32)
        par16[:, woff[l]:woff[l] + fhs[l]] = W.T.astype(np.float16)
        par16[:fhs[l], 320 + l] = aW[0, fhs[l]:2 * fhs[l]].astype(np.float16)
        parf[:fhs[l], l] = b.reshape(-1)
    in_maps = []
    for c in range(N_CORES):
        blk = adj[c * NL:(c + 1) * NL, :].astype(ml_dtypes.float8_e4m3)
        # [NL, N] -> (m, h, q, kp, i, p) -> [p, m, kp, i, h, q]
        adjt = blk.reshape(NT, 2, 64, KP, 2, 128).transpose(5, 0, 3, 4, 1, 2)
        m = {
            "adjt": np.ascontiguousarray(adjt),
            "xt_local": np.ascontiguousarray(
                x[c * NL:(c + 1) * NL, :].T.astype(np.float16)),
            "par16": par16,
            "parf": parf,
        }
        in_maps.append(m)
    return in_maps


def kernel(x, adj, W0, b0, aW0, ab0, W1, b1, aW1, ab1, W2, b2, aW2, ab2):
    inputs = dict(x=x, adj=adj, W0=W0, b0=b0, aW0=aW0, ab0=ab0,
                  W1=W1, b1=b1, aW1=aW1, ab1=ab1, W2=W2, b2=b2, aW2=aW2, ab2=ab2)
    ab = [float(np.asarray(inputs[f"ab{l}"]).reshape(-1)[0]) for l in range(3)]
    nc = _get_program(ab)
    in_maps = _make_in_maps(inputs)
    res = run_bass_kernel_spmd(nc, in_maps, list(range(N_CORES)))
    out = np.concatenate([res.results[c]["out"] for c in range(N_CORES)], axis=0)
    return out.astype(np.float32)
